# revision 1
# baseline (speedup 1.0000x reference)
"""NT-Xent contrastive loss on 8 Trainium2 NeuronCores.

reference math:
  z = concat(h1, h2)            [8192, 512]
  zn = z / max(||z||, eps)      row-normalized
  sim = zn @ zn.T               [8192, 8192], diag masked to -inf
  loss_i = -pos_i/T + log(sum_j!=i exp(sim_ij/T)),  T = 0.5
  out = mean_i(loss_i)

Sharding: data-parallel over the 8192 sim rows -> 1024 rows per core.
Each core gets the full zn^T (bf16) as the moving GEMM operand plus its
own column-block as the stationary operand; the diag (self) and positive
terms are computed from per-core row data so the SPMD program is
identical across cores (only input data differs). No collectives.

sim/T is in [-2, 2], so exp needs no max-subtraction; the row sum is
computed with the scalar engine's fused exp+accumulate directly from
PSUM, and the diag is removed by subtracting exp(self/T).
"""

from contextlib import ExitStack

import ml_dtypes
import numpy as np

import concourse.bass as bass
import concourse.tile as tile
from concourse import mybir
from concourse.bass_utils import run_bass_kernel_spmd

N_CORES = 8
B = 4096
N = 2 * B          # 8192 total rows
D = 512            # feature dim
RPC = N // N_CORES  # 1024 rows per core
MT = RPC // 128    # 8 m-tiles per core
KC = D // 128      # 4 contraction chunks
NGW = 2048         # psum tile width (4 banks)
NG = N // NGW      # 4 n-groups
MM_N = 512         # moving-operand width per matmul (this walrus caps
                   # s3d3_mm_num_elements at 512 even for bf16)
# uniform column groups measured fastest (narrow head/tail groups added
# more ACT/gate overhead than the DMA-head time they saved)
COLS = [(0, 2048), (2048, 2048), (4096, 2048), (6144, 2048)]
T_INV = 2.0        # 1 / temperature
EPS = 1e-8

BF16 = ml_dtypes.bfloat16
FP32 = mybir.dt.float32
MBF16 = mybir.dt.bfloat16


def _patch_sem_range_clear():
    """This walrus build rejects the EVENT_SEMAPHORE_RANGE_CLEAR raw-ISA
    struct ("ISA wrong length") that TileContext emits in its epilogue.
    Skip emitting it (the bookkeeping is kept); semaphores are reset at
    NEFF load, and the kernel runs once per load."""
    if getattr(bass.Bass, "_sem_clear_patched", False):
        return

    def clear_and_free_semaphores(self, sems):
        if not sems:
            return
        sem_nums = [
            sem.num if isinstance(sem, bass.SemaphoreHandle) else sem
            for sem in sems
        ]
        self._state.prepend_free_semaphores(sem_nums)
        for poison_set in self._tile_sem_poison_stack:
            poison_set.update(sem_nums)

    bass.Bass.clear_and_free_semaphores = clear_and_free_semaphores
    bass.Bass._sem_clear_patched = True


def _build_program():
    _patch_sem_range_clear()
    nc = bass.Bass("TRN2", target_bir_lowering=False, debug=False,
                   num_devices=N_CORES)

    rhs_d = nc.dram_tensor("rhs4", [KC, 128, N], MBF16,
                           kind="ExternalInput").ap()
    lhsT_d = nc.dram_tensor("lhst4", [KC, 128, RPC], MBF16,
                            kind="ExternalInput").ap()
    zrow_d = nc.dram_tensor("zrow", [128, MT, D], MBF16,
                            kind="ExternalInput").ap()
    zpos_d = nc.dram_tensor("zpos", [128, MT, D], MBF16,
                            kind="ExternalInput").ap()
    loss_d = nc.dram_tensor("loss", [128, MT], FP32,
                            kind="ExternalOutput").ap()

    with tile.TileContext(nc) as tc, ExitStack() as ctx:
        # All tiles are persistent (allocated once, never pool-recycled):
        # pool slot reuse emits multi-semaphore alloc waits, and this
        # toolchain's walrus accepts only ONE sync wait per queue
        # instruction. Per-instruction deps keep every wait count <= 1.
        const = ctx.enter_context(tc.tile_pool(name="const", bufs=1))
        psum = ctx.enter_context(
            tc.tile_pool(name="psum", bufs=1, space=bass.MemorySpace.PSUM))
        stats = ctx.enter_context(tc.tile_pool(name="stats", bufs=1))

        rhs_t = const.tile([128, KC, N], MBF16)
        lhsT_t = const.tile([128, KC, RPC], MBF16)
        zrow_t = const.tile([128, MT, D], MBF16)
        zpos_t = const.tile([128, MT, D], MBF16)

        for kc in range(KC):
            # split so the first m-tiles' weights land before the rest
            nc.sync.dma_start(lhsT_t[:, kc, 0:256], lhsT_d[kc, :, 0:256])
            nc.sync.dma_start(lhsT_t[:, kc, 256:RPC], lhsT_d[kc, :, 256:RPC])
        for lo, w in COLS:
            for kc in range(KC):
                nc.sync.dma_start(rhs_t[:, kc, lo:lo + w],
                                  rhs_d[kc, :, lo:lo + w])
        nc.sync.dma_start(zrow_t[:], zrow_d[:])
        nc.sync.dma_start(zpos_t[:], zpos_d[:])

        # exp-row-sum partials: one [128,1] slot per (m, ngroup)
        ss = stats.tile([128, MT, len(COLS)], FP32)
        self_s = stats.tile([128, MT], FP32)
        pos_s = stats.tile([128, MT], FP32)

        # absorb zrow/zpos DMA waits into single-wait DVE copies so the
        # tensor_tensor_reduce ops below carry at most one wait
        sliver = stats.tile([128, 2], FP32)
        nc.vector.tensor_copy(sliver[:, 0:1], zrow_t[:, 0, 0:1])
        nc.vector.tensor_copy(sliver[:, 1:2], zpos_t[:, 0, 0:1])

        # self & positive dot products from row-major block data
        so = stats.tile([128, D], FP32)
        po = stats.tile([128, D], FP32)
        for m in range(MT):
            nc.vector.tensor_mul(so[:], zrow_t[:, m, :], zrow_t[:, m, :])
            nc.vector.tensor_reduce(self_s[:, m:m + 1], so[:],
                                    axis=mybir.AxisListType.X,
                                    op=mybir.AluOpType.add)
            nc.vector.tensor_mul(po[:], zrow_t[:, m, :], zpos_t[:, m, :])
            nc.vector.tensor_reduce(pos_s[:, m:m + 1], po[:],
                                    axis=mybir.AxisListType.X,
                                    op=mybir.AluOpType.add)

        # two persistent psum tiles, ping-ponged manually
        ps_a = psum.tile([128, NGW], FP32)
        ps_b = psum.tile([128, NGW], FP32)
        ps_tiles = [ps_a, ps_b]
        # per-group gate landing pad (distinct column per group -> no deps
        # between gates)
        gate_out = stats.tile([128, len(COLS) * MT], FP32)

        # main GEMM + fused exp row-sums (exp is done in-place in PSUM;
        # only the per-row accumulator output is kept)
        gi = 0
        for ci, (base, w) in enumerate(COLS):
            # absorb this group's rhs-chunk DMA waits (and, on the first
            # group, the lhsT DMA waits) into dummy weight loads on PE
            for kc in range(KC):
                if ci == 0:
                    nc.tensor.ldweights(lhsT_t[:, kc, 0:128])
                nc.tensor.ldweights(rhs_t[:, kc, base:base + 128])
            for m in range(MT):
                ps = ps_tiles[gi % 2]
                for kc in range(KC):
                    for n in range(w // MM_N):
                        nc.tensor.matmul(
                            ps[:, n * MM_N:(n + 1) * MM_N],
                            lhsT_t[:, kc, m * 128:(m + 1) * 128],
                            rhs_t[:, kc, base + n * MM_N:base + (n + 1) * MM_N],
                            start=(kc == 0), stop=(kc == KC - 1))
                # gate: a tiny ACT read of the last-written psum column
                # absorbs the PE wait, so the exp below carries only its
                # (single) same-engine wait
                nc.scalar.activation(
                    gate_out[:, gi:gi + 1], ps[:, w - 1:w],
                    mybir.ActivationFunctionType.Copy)
                nc.scalar.activation(
                    ps[:, 0:w], ps[:, 0:w],
                    mybir.ActivationFunctionType.Exp,
                    scale=T_INV, accum_out=ss[:, m, ci:ci + 1])
                gi += 1

        # loss = ln(S - exp(self/T)) - pos/T
        stot = stats.tile([128, MT], FP32)
        nc.vector.tensor_reduce(stot[:], ss[:], axis=mybir.AxisListType.X,
                                op=mybir.AluOpType.add)
        eself = stats.tile([128, MT], FP32)
        nc.scalar.activation(eself[:], self_s[:],
                             mybir.ActivationFunctionType.Exp, scale=T_INV)
        masked = stats.tile([128, MT], FP32)
        nc.vector.tensor_sub(masked[:], stot[:], eself[:])
        lnv = stats.tile([128, MT], FP32)
        nc.scalar.activation(lnv[:], masked[:],
                             mybir.ActivationFunctionType.Ln)
        pos2 = stats.tile([128, MT], FP32)
        nc.scalar.mul(pos2[:], pos_s[:], T_INV)
        lossv = stats.tile([128, MT], FP32)
        nc.vector.tensor_sub(lossv[:], lnv[:], pos2[:])
        # gpsimd DMAs ride the (otherwise unused) SWDGE lanes: no HW-queue
        # predecessor wait, so this carries only the DVE producer dep
        nc.gpsimd.dma_start(loss_d[:], lossv[:])

    _split_multi_waits(nc)
    return nc


def _split_multi_waits(nc):
    """walrus here accepts only one sync wait per instruction; hoist extra
    waits onto standalone wait-only EventSemaphore carriers."""
    for f in nc.m.functions:
        for b in f.blocks:
            new_insts = []
            for inst in b.instructions:
                si = inst.sync_info
                if si is not None and si.on_wait and len(si.on_wait) > 1:
                    waits = list(si.on_wait)
                    for w in waits[:-1]:
                        carrier = mybir.InstEventSemaphore(
                            name=nc.get_next_instruction_name(),
                            engine=inst.engine,
                            ins=[], outs=[],
                            sync_info=mybir.SyncInfo(on_wait=[w],
                                                     on_update=[]),
                        )
                        new_insts.append(carrier)
                    inst.sync_info = mybir.SyncInfo(on_wait=[waits[-1]],
                                                    on_update=si.on_update)
                new_insts.append(inst)
            b.instructions = new_insts


_NC_CACHE = None


def _get_program():
    global _NC_CACHE
    if _NC_CACHE is None:
        _NC_CACHE = _build_program()
    return _NC_CACHE


def _prep_inputs(aug_hidden1, aug_hidden2):
    h1 = np.asarray(aug_hidden1, dtype=np.float32)
    h2 = np.asarray(aug_hidden2, dtype=np.float32)
    z = np.concatenate([h1, h2], axis=0)
    norms = np.sqrt(np.sum(z * z, axis=1, keepdims=True))
    zn = z / np.maximum(norms, EPS)

    znb = zn.astype(BF16)                       # one rounding, shared by all views
    rhs4 = np.ascontiguousarray(znb.T).reshape(KC, 128, N)

    in_maps = []
    for c in range(N_CORES):
        r0 = c * RPC
        lhsT4 = np.ascontiguousarray(rhs4[:, :, r0:r0 + RPC])
        zrow = np.ascontiguousarray(
            znb[r0:r0 + RPC].reshape(MT, 128, D).transpose(1, 0, 2))
        idx = (np.arange(r0, r0 + RPC) + B) % N
        zpos = np.ascontiguousarray(
            znb[idx].reshape(MT, 128, D).transpose(1, 0, 2))
        in_maps.append({
            "rhs4": rhs4,
            "lhst4": lhsT4,
            "zrow": zrow,
            "zpos": zpos,
        })
    return in_maps


def _finish(results):
    rows = np.empty((N_CORES, MT, 128), dtype=np.float32)
    for c in range(N_CORES):
        rows[c] = results[c]["loss"].T        # [MT, 128]
    total = rows.reshape(-1).astype(np.float64).mean()
    return np.float32(total)


def run(inputs, trace=False):
    """Returns (loss_scalar, exec_time_ns_or_None)."""
    nc = _get_program()
    in_maps = _prep_inputs(inputs["aug_hidden1"], inputs["aug_hidden2"])
    res = run_bass_kernel_spmd(nc, in_maps, list(range(N_CORES)), trace=trace)
    return _finish(res.results), res.exec_time_ns


def kernel(aug_hidden1, aug_hidden2):
    out, _ = run({"aug_hidden1": aug_hidden1, "aug_hidden2": aug_hidden2})
    return out



# revision 7
# speedup vs baseline: 1.5804x; 1.5804x over previous
"""NT-Xent contrastive loss on 8 Trainium2 NeuronCores (fp8 GEMM).

reference math:
  z = concat(h1, h2)            [8192, 512]
  zn = z / max(||z||, eps)      row-normalized
  sim = zn @ zn.T               [8192, 8192], diag masked to -inf
  loss_i = -pos_i/T + log(sum_j!=i exp(sim_ij/T)),  T = 0.5
  out = mean_i(loss_i)

Sharding: data-parallel over the 8192 sim rows -> 1024 rows per core.
Each core receives the full zn^T in fp8 (scaled by S8=16) with its columns
ROTATED so the core's own 1024 rows land in columns 0:1024 — the GEMM
stationary operand is then a slice of the same SBUF tile on every core,
and the SPMD program is identical across cores. No collectives.

GEMM runs in fp8 (e4m3) DoubleRow perf mode: each matmul contracts 256
rows (2 k-tiles) per pass, 2x the bf16 rate. PSUM holds S8^2*sim, and the
scalar engine's fused exp+accumulate rescales by T_INV/S8^2 on the fly.
The diag (self) and positive terms come from bf16 per-core row data so
the fp8 error stays off the pos term.
"""

from contextlib import ExitStack

import ml_dtypes
import numpy as np

import concourse.bass as bass
import concourse.tile as tile
from concourse import mybir
from concourse.bass_utils import run_bass_kernel_spmd

N_CORES = 8
B = 4096
N = 2 * B          # 8192 total rows
D = 512            # feature dim
RPC = N // N_CORES  # 1024 rows per core
MT = RPC // 128    # 8 m-tiles per core
KC2 = 2            # DoubleRow contraction chunks (256 rows each)
NGW = 2048         # psum tile width (4 banks)
MM_N = 512         # moving-operand width per matmul
COLS = [(0, 2048), (2048, 2048), (4096, 2048), (6144, 2048)]
T_INV = 2.0        # 1 / temperature
EPS = 1e-8
S8 = 16.0          # fp8 pre-scale: fp8 stores zn*S8, PSUM holds S8^2*sim
EXP_SCALE = T_INV / (S8 * S8)          # 0.0078125 (exact)

BF16 = ml_dtypes.bfloat16
FP32 = mybir.dt.float32
MBF16 = mybir.dt.bfloat16
MF8 = mybir.dt.float8e4
F16 = mybir.dt.float16
F8NP = mybir.dt.np(mybir.dt.float8e4)
DR = mybir.MatmulPerfMode.DoubleRow


def _patch_sem_range_clear():
    """This walrus build rejects the EVENT_SEMAPHORE_RANGE_CLEAR raw-ISA
    struct ("ISA wrong length") that TileContext emits in its epilogue.
    Skip emitting it (the bookkeeping is kept); semaphores are reset at
    NEFF load, and the kernel runs once per load."""
    if getattr(bass.Bass, "_sem_clear_patched", False):
        return

    def clear_and_free_semaphores(self, sems):
        if not sems:
            return
        sem_nums = [
            sem.num if isinstance(sem, bass.SemaphoreHandle) else sem
            for sem in sems
        ]
        self._state.prepend_free_semaphores(sem_nums)
        for poison_set in self._tile_sem_poison_stack:
            poison_set.update(sem_nums)

    bass.Bass.clear_and_free_semaphores = clear_and_free_semaphores
    bass.Bass._sem_clear_patched = True


def _build_program():
    _patch_sem_range_clear()
    nc = bass.Bass("TRN2", target_bir_lowering=False, debug=False,
                   num_devices=N_CORES)

    rhs_d = nc.dram_tensor("rhs8", [KC2, 128, 2, N], MF8,
                           kind="ExternalInput").ap()
    zrow_d = nc.dram_tensor("zrow", [128, MT, D], MBF16,
                            kind="ExternalInput").ap()
    zpos_d = nc.dram_tensor("zpos", [128, MT, D], MBF16,
                            kind="ExternalInput").ap()
    loss_d = nc.dram_tensor("loss", [128, MT], FP32,
                            kind="ExternalOutput").ap()

    with tile.TileContext(nc) as tc, ExitStack() as ctx:
        # All tiles are persistent (allocated once, never pool-recycled):
        # pool slot reuse emits multi-semaphore alloc waits, and this
        # toolchain's walrus accepts only ONE sync wait per queue
        # instruction (extra waits are hoisted by _split_multi_waits).
        const = ctx.enter_context(tc.tile_pool(name="const", bufs=1))
        psum = ctx.enter_context(
            tc.tile_pool(name="psum", bufs=1, space=bass.MemorySpace.PSUM))
        stats = ctx.enter_context(tc.tile_pool(name="stats", bufs=1))

        rhs_t = const.tile([128, KC2, 2, N], MF8)
        zrow_t = const.tile([128, MT, D], MBF16)
        zpos_t = const.tile([128, MT, D], MBF16)

        # DMA order = arrival order: the first two column groups feed the
        # first ~27us of GEMM; zrow/zpos (pos/self terms, consumed
        # mid-kernel by DVE) ride between groups 1 and 2.
        for ci in (0, 1):
            lo, w = COLS[ci]
            for kc2 in range(KC2):
                nc.sync.dma_start(rhs_t[:, kc2, :, lo:lo + w],
                                  rhs_d[kc2, :, :, lo:lo + w])
        nc.sync.dma_start(zrow_t[:], zrow_d[:])
        nc.sync.dma_start(zpos_t[:], zpos_d[:])
        for ci in (2, 3):
            lo, w = COLS[ci]
            for kc2 in range(KC2):
                nc.sync.dma_start(rhs_t[:, kc2, :, lo:lo + w],
                                  rhs_d[kc2, :, :, lo:lo + w])

        # exp-row-sum partials: one [128,1] slot per (m, ngroup)
        ss = stats.tile([128, MT, len(COLS)], FP32)
        self_s = stats.tile([128, MT], FP32)
        pos_s = stats.tile([128, MT], FP32)

        # scratch products for the pos/self dot products; f16 SBUF so the
        # DVE runs its 16-bit 2x perf mode
        so = stats.tile([128, D], F16)
        po = stats.tile([128, D], F16)

        # two persistent psum tiles, ping-ponged manually
        ps_a = psum.tile([128, NGW], FP32)
        ps_b = psum.tile([128, NGW], FP32)
        ps_tiles = [ps_a, ps_b]

        def emit_pos_self():
            # self & positive dot products from bf16 row-major block data
            # (vector engine; idle during the GEMM otherwise)
            for m in range(MT):
                nc.vector.tensor_mul(so[:], zrow_t[:, m, :], zrow_t[:, m, :])
                nc.vector.tensor_reduce(self_s[:, m:m + 1], so[:],
                                        axis=mybir.AxisListType.X,
                                        op=mybir.AluOpType.add)
                nc.vector.tensor_mul(po[:], zrow_t[:, m, :], zpos_t[:, m, :])
                nc.vector.tensor_reduce(pos_s[:, m:m + 1], po[:],
                                        axis=mybir.AxisListType.X,
                                        op=mybir.AluOpType.add)

        # main GEMM + fused exp row-sums. PE sweeps 512-col fp8 DoubleRow
        # matmuls (contracting 256 rows per pass); the stationary operand
        # is the core's own row block = columns 0:1024 of the same rhs
        # tile (column rotation puts it there on every core).
        gi = 0
        for ci, (base, w) in enumerate(COLS):
            # absorb this group's rhs-chunk DMA waits into dummy weight
            # loads on PE
            for kc2 in range(KC2):
                nc.tensor.ldweights(rhs_t[:, kc2, :, base:base + 128],
                                    perf_mode=DR)
            for m in range(MT):
                ps = ps_tiles[gi % 2]
                for kc2 in range(KC2):
                    for n in range(w // MM_N):
                        nc.tensor.matmul(
                            ps[:, n * MM_N:(n + 1) * MM_N],
                            rhs_t[:, kc2, :, m * 128:(m + 1) * 128],
                            rhs_t[:, kc2, :, base + n * MM_N:
                                  base + (n + 1) * MM_N],
                            start=(kc2 == 0), stop=(kc2 == KC2 - 1),
                            perf_mode=DR)
                # scalar-engine exp, in place in PSUM; only the per-row
                # accumulator output is kept. The single PE-sem wait
                # rides on the exp itself (no gate).
                nc.scalar.activation(
                    ps[:, 0:w], ps[:, 0:w],
                    mybir.ActivationFunctionType.Exp,
                    scale=EXP_SCALE, accum_out=ss[:, m, ci:ci + 1])
                gi += 1
            if ci == 1:
                # zrow/zpos have landed by now; DVE is between exp tiles
                emit_pos_self()

        # loss = ln(S - exp(self/T)) - pos/T
        stot = stats.tile([128, MT], FP32)
        nc.vector.tensor_reduce(stot[:], ss[:], axis=mybir.AxisListType.X,
                                op=mybir.AluOpType.add)
        eself = stats.tile([128, MT], FP32)
        nc.scalar.activation(eself[:], self_s[:],
                             mybir.ActivationFunctionType.Exp, scale=T_INV)
        masked = stats.tile([128, MT], FP32)
        nc.vector.tensor_sub(masked[:], stot[:], eself[:])
        lnv = stats.tile([128, MT], FP32)
        nc.scalar.activation(lnv[:], masked[:],
                             mybir.ActivationFunctionType.Ln)
        pos2 = stats.tile([128, MT], FP32)
        nc.scalar.mul(pos2[:], pos_s[:], T_INV)
        lossv = stats.tile([128, MT], FP32)
        nc.vector.tensor_sub(lossv[:], lnv[:], pos2[:])
        # gpsimd DMAs ride the (otherwise unused) SWDGE lanes: no HW-queue
        # predecessor wait, so this carries only the DVE producer dep
        nc.gpsimd.dma_start(loss_d[:], lossv[:])

    _split_multi_waits(nc)
    return nc


def _split_multi_waits(nc):
    """walrus here accepts only one sync wait per instruction; hoist extra
    waits onto standalone wait-only EventSemaphore carriers."""
    for f in nc.m.functions:
        for b in f.blocks:
            new_insts = []
            for inst in b.instructions:
                si = inst.sync_info
                if si is not None and si.on_wait and len(si.on_wait) > 1:
                    waits = list(si.on_wait)
                    for w in waits[:-1]:
                        carrier = mybir.InstEventSemaphore(
                            name=nc.get_next_instruction_name(),
                            engine=inst.engine,
                            ins=[], outs=[],
                            sync_info=mybir.SyncInfo(on_wait=[w],
                                                     on_update=[]),
                        )
                        new_insts.append(carrier)
                    inst.sync_info = mybir.SyncInfo(on_wait=[waits[-1]],
                                                    on_update=si.on_update)
                new_insts.append(inst)
            b.instructions = new_insts


_NC_CACHE = None


def _get_program():
    global _NC_CACHE
    if _NC_CACHE is None:
        _NC_CACHE = _build_program()
    return _NC_CACHE


def _prep_inputs(aug_hidden1, aug_hidden2):
    h1 = np.asarray(aug_hidden1, dtype=np.float32)
    h2 = np.asarray(aug_hidden2, dtype=np.float32)
    z = np.concatenate([h1, h2], axis=0)
    norms = np.sqrt(np.sum(z * z, axis=1, keepdims=True))
    zn = z / np.maximum(norms, EPS)

    znb = zn.astype(BF16)                  # bf16 rows for pos/self terms
    zn8t = np.ascontiguousarray((zn.T * S8).astype(np.float32)).astype(F8NP)
    # [512, 8192] -> per-core rotated [KC2, 128, 2, N]

    in_maps = []
    for c in range(N_CORES):
        r0 = c * RPC
        perm = (np.arange(N) + r0) % N
        rot = zn8t[:, perm]                          # [512, N]
        rhs8 = np.ascontiguousarray(
            rot.reshape(KC2, 2, 128, N).transpose(0, 2, 1, 3))
        zrow = np.ascontiguousarray(
            znb[r0:r0 + RPC].reshape(MT, 128, D).transpose(1, 0, 2))
        idx = (np.arange(r0, r0 + RPC) + B) % N
        zpos = np.ascontiguousarray(
            znb[idx].reshape(MT, 128, D).transpose(1, 0, 2))
        in_maps.append({
            "rhs8": rhs8,
            "zrow": zrow,
            "zpos": zpos,
        })
    return in_maps


def _finish(results):
    rows = np.empty((N_CORES, MT, 128), dtype=np.float32)
    for c in range(N_CORES):
        rows[c] = results[c]["loss"].T        # [MT, 128]
    total = rows.reshape(-1).astype(np.float64).mean()
    return np.float32(total)


def run(inputs, trace=False):
    """Returns (loss_scalar, exec_time_ns_or_None)."""
    nc = _get_program()
    in_maps = _prep_inputs(inputs["aug_hidden1"], inputs["aug_hidden2"])
    res = run_bass_kernel_spmd(nc, in_maps, list(range(N_CORES)), trace=trace)
    return _finish(res.results), res.exec_time_ns


def kernel(aug_hidden1, aug_hidden2):
    out, _ = run({"aug_hidden1": aug_hidden1, "aug_hidden2": aug_hidden2})
    return out
